# revision 1
# baseline (speedup 1.0000x reference)
"""Bass/Tile kernel for a single causal attention head on 8 trn2 NeuronCores.

Problem: input [8, 2048, 768], Wq/Wk/Wv [768, 64] ->
  O = softmax(causal(Q K^T)/sqrt(64)) V, per batch.  [8, 2048, 64]

Sharding: data-parallel over batch; core b handles batch b. Weights replicated.

Per-core dataflow:
  1. x [2048, 768] -> PE-transpose (exact fp32) -> xT [768, 2048] in SBUF.
  2. Projections with concatenated stationary weights, so one 128-wide
     matmul yields two 64-row outputs:
       [Wq|Wk]^T xT -> psum [128, 512]: rows 0-63 Q^T, rows 64-127 K^T
       [Wv|Wq]^T xT -> psum [128, 512]: rows 0-63 V^T, rows 64-127 Q^T (dup)
     One full-width copy each into QK_sb / VQ_sb.  V^T tiles are
     PE-transposed back into V_aug tiles [128, 65] whose ones column makes
     the O^T matmul emit softmax denominators for free (row 64).
  3. For each 512-wide query range r:
       full k-tiles (j < 4r) in pairs: S^T pair-psum [128, 2x512] with
         lhsT=K^T_j, rhs=Q^T (both on partitions 64-127); one exp per pair
         (ScalarE, 1/8 scale fused).
       diagonal k-tiles j=4r..4r+3, widths 512/384/256/128: three packed
         into a [128, 1024] psum + one [128, 256]; after exp, the invalid
         (q < k) halves of the diagonal 128x128 blocks are zeroed by
         GPSIMD affine_select on SBUF.
       O^T [65, 512] accumulates V_aug_j^T @ expS^T over j in PSUM.
  4. O^T+rowsum PE-transposed back to [128, 65]; out = O * (1/rowsum); DMA.

Matmul inputs are stored as float32r (single-pass full-rate fp32 PE mode;
producing copies round to f32r precision; ~3e-4 scale-relative output
error).  ATTN_MM_DTYPE=float32 selects the exact two-pass fp32 mode
(4 cyc/row, ~2.4e-6 error).  ATTN_X=bf16 selects a bf16 x-path (host-cast
+ xbar DMA transpose, ~2.4e-3 error).
"""

import os
import numpy as np

import concourse.tile as tile
from concourse import bacc, mybir
from concourse.bass_utils import run_bass_kernel_spmd
from concourse.masks import make_identity

P = 128
N = 2048
D = 768
H = 64
NT = N // P   # 16 n-tiles
DT = D // P   # 6 d-tiles
W = 512       # q-range width
QR = N // W   # 4 q-ranges
F32 = mybir.dt.float32

MM_F32R = os.environ.get("ATTN_MM_DTYPE", "float32r") == "float32r"
MMDT = mybir.dt.float32r if MM_F32R else F32
# bf16 x-path: host-cast x to bf16, xbar DMA-transpose straight from DRAM
# (removes all PE transposes + PSUM->SBUF copies); projections run in bf16.
BF16_X = os.environ.get("ATTN_X", "f32") == "bf16"
XDT = mybir.dt.bfloat16 if BF16_X else MMDT

# Three of the four diagonal tiles (widths 512, 384, 128) pack into one
# [128, 1024] psum (bank0: 512 | bank1: 384+128); the 256-wide one (jj=2)
# gets its own narrow tile.  No matmul output crosses a 512-col bank.
DIAG_PACK = {0: (0, 512), 1: (512, 384), 3: (896, 128)}  # jj -> (off, width)
DIAG_TOT = 1024


def build_kernel(reps=1):
    nc = bacc.Bacc(name="attn_head")
    x_d = nc.dram_tensor("x", [N, D], XDT if BF16_X else F32,
                         kind="ExternalInput")
    wq_d = nc.dram_tensor("Wq", [D, H], F32, kind="ExternalInput")
    wk_d = nc.dram_tensor("Wk", [D, H], F32, kind="ExternalInput")
    wv_d = nc.dram_tensor("Wv", [D, H], F32, kind="ExternalInput")
    out_d = nc.dram_tensor("out", [N, H], F32, kind="ExternalOutput")

    Exp = mybir.ActivationFunctionType.Exp

    with tile.TileContext(nc) as tc:
        with (
            tc.tile_pool(name="persist", bufs=1) as persist,
            tc.tile_pool(name="xload", bufs=16) as xload,
            tc.tile_pool(name="work", bufs=3) as work,
            tc.tile_pool(name="psum", bufs=1, space="PSUM") as psum,
        ):
            # warm the ACT exp table while DMAs run
            dummy = persist.tile([P, 1], F32)
            nc.vector.memset(dummy[:], 0.0)
            nc.scalar.activation(dummy[:], dummy[:], Exp)

            ident = persist.tile([P, P], F32)
            make_identity(nc, ident[:])

            ones_col = persist.tile([P, 1], F32)
            nc.vector.memset(ones_col[:], 1.0)

            if MM_F32R:
                ident_r = persist.tile([P, P], MMDT)
                nc.vector.tensor_copy(out=ident_r[:], in_=ident[:])
            else:
                ident_r = ident

            xT = persist.tile([P, DT, N], XDT)      # x^T: [d%128, d//128, n]
            QK_sb = persist.tile([P, N], MMDT)      # rows 0-63 Q^T, 64-127 K^T
            VQ_sb = persist.tile([P, N], MMDT)      # rows 0-63 V^T, 64-127 Q^T
            Vb = persist.tile([P, NT, H + 1], MMDT)  # V tiles + ones col
            nc.vector.tensor_copy(
                out=Vb[:, :, H],
                in_=ones_col[:, 0].to_broadcast((P, NT)),
            )

            # weights as [128, 6, 64]: partition = d%128, tile = d//128.
            # Concatenated pairs [Wq|Wk] and [Wv|Wq] make 128-wide stationary
            # operands: one projection matmul computes two 64-row outputs.
            w_raw = {}
            w_dma = []
            for wname, dram in (("q", wq_d), ("k", wk_d), ("v", wv_d)):
                w_raw[wname] = persist.tile([P, DT, H], F32,
                                            name=f"wraw_{wname}")
                w_dma.append((w_raw[wname], dram))
            w_qk = persist.tile([P, DT, 2 * H], XDT)
            w_vq = persist.tile([P, DT, 2 * H], XDT)

            def load_weights():
                for w_t, dram in w_dma:
                    nc.sync.dma_start(
                        out=w_t[:],
                        in_=dram[:, :].rearrange("(t p) h -> p t h", p=P),
                    )
                nc.vector.tensor_copy(out=w_qk[:, :, 0:H], in_=w_raw["q"][:])
                nc.vector.tensor_copy(out=w_qk[:, :, H:], in_=w_raw["k"][:])
                nc.vector.tensor_copy(out=w_vq[:, :, 0:H], in_=w_raw["v"][:])
                nc.vector.tensor_copy(out=w_vq[:, :, H:], in_=w_raw["q"][:])

            for rep in range(reps):
                # ---- x load + transpose to xT ------------------------------
                if BF16_X:
                    # x arrives bf16: xbar DMA-transpose straight from DRAM,
                    # one [2048, 128] -> [128, 2048] transpose per d-block
                    for d_i in range(DT):
                        nc.sync.dma_start(
                            out=xT[:, d_i, :],
                            in_=x_d[:, d_i * P:(d_i + 1) * P],
                            transpose=True,
                        )
                    if rep == 0:
                        load_weights()
                else:
                    # first group (tiles 0-3) split into d-halves so the
                    # first PE transposes start after ~4 small DMAs
                    xts = []
                    for nt in range(NT):
                        xt = xload.tile([P, D], F32, tag="x",
                                        name=f"x_{rep}_{nt}")
                        xts.append(xt)
                    for nt in range(4):
                        nc.sync.dma_start(
                            out=xts[nt][:, 0:D // 2],
                            in_=x_d[nt * P:(nt + 1) * P, 0:D // 2])
                    for nt in range(4):
                        nc.sync.dma_start(
                            out=xts[nt][:, D // 2:],
                            in_=x_d[nt * P:(nt + 1) * P, D // 2:])
                    if rep == 0:
                        load_weights()
                    for nt in range(4, NT):
                        nc.sync.dma_start(
                            out=xts[nt][:, 0:D // 2],
                            in_=x_d[nt * P:(nt + 1) * P, 0:D // 2])
                        nc.sync.dma_start(
                            out=xts[nt][:, D // 2:],
                            in_=x_d[nt * P:(nt + 1) * P, D // 2:])

                    for g in range(QR):
                        for dp in range(DT // 2):
                            pt = psum.tile([P, 2, W], F32, tag="mm", bufs=2)
                            for u in range(2):
                                d_i = 2 * dp + u
                                for i in range(4):
                                    nc.tensor.transpose(
                                        pt[:, u, i * P:(i + 1) * P],
                                        xts[g * 4 + i][:,
                                                       d_i * P:(d_i + 1) * P],
                                        ident[:],
                                    )
                            base = g * W
                            if (g * 3 + dp) % 4 == 3:
                                nc.scalar.copy(
                                    out=xT[:, 2 * dp:2 * dp + 2,
                                           base:base + W],
                                    in_=pt[:])
                            else:
                                nc.vector.tensor_copy(
                                    out=xT[:, 2 * dp:2 * dp + 2,
                                           base:base + W],
                                    in_=pt[:])

                # ---- projections (packed via concatenated weights) ---------
                for r in range(QR):
                    ns = slice(r * W, (r + 1) * W)
                    pqk = psum.tile([P, W], F32, tag="proj", bufs=2)
                    pvq = psum.tile([P, W], F32, tag="proj", bufs=2)
                    for d_i in range(DT):
                        kw = dict(start=(d_i == 0), stop=(d_i == DT - 1))
                        rhs = xT[:, d_i, ns]
                        nc.tensor.matmul(pqk[:], w_qk[:, d_i], rhs, **kw)
                        nc.tensor.matmul(pvq[:], w_vq[:, d_i], rhs, **kw)
                    nc.vector.tensor_copy(out=QK_sb[:, ns], in_=pqk[:])
                    nc.vector.tensor_copy(out=VQ_sb[:, ns], in_=pvq[:])
                    # V^T tiles -> V_aug [128, 65] per k-tile (batched copy)
                    pvt = psum.tile([P, 4, H], MMDT, tag="proj", bufs=2)
                    for i in range(4):
                        nc.tensor.transpose(
                            pvt[:, i, :],
                            VQ_sb[0:H, r * W + i * P:r * W + (i + 1) * P],
                            ident_r[:H, :H],
                        )
                    nc.vector.tensor_copy(out=Vb[:, 4 * r:4 * r + 4, 0:H],
                                          in_=pvt[:])

                # ---- attention per q-range ---------------------------------
                KTd = QK_sb[H:P, :]   # K^T on partitions 64-127
                QTd = VQ_sb[H:P, :]   # Q^T duplicate on partitions 64-127
                for r in range(QR):
                    po = psum.tile([H + 1, W], F32, tag="po", bufs=2)
                    qs = slice(r * W, (r + 1) * W)
                    last_r = (r == QR - 1)

                    def do_pairs(first, last):
                        for jp in range(2 * r):
                            ps2 = psum.tile([P, 2, W], F32, tag="mm", bufs=2,
                                            name=f"ps2_{rep}_{r}_{jp}")
                            es2 = work.tile([P, 2, W], MMDT, tag="es2", bufs=6,
                                            name=f"es2_{rep}_{r}_{jp}")
                            for u in range(2):
                                j = 2 * jp + u
                                nc.tensor.matmul(
                                    ps2[:, u, :], KTd[:, j * P:(j + 1) * P],
                                    QTd[:, qs], start=True, stop=True,
                                )
                            nc.scalar.activation(es2[:], ps2[:], Exp,
                                                 scale=0.125)
                            for u in range(2):
                                j = 2 * jp + u
                                nc.tensor.matmul(
                                    po[:], Vb[:, j, :], es2[:, u, :],
                                    start=(first and j == 0),
                                    stop=(last and jp == 2 * r - 1 and u == 1),
                                    skip_group_check=True,
                                )

                    if not last_r:
                        do_pairs(first=True, last=False)

                    # diagonal k-tiles: 3 packed in [128, 1024] + 1 [128, 256]
                    psd = psum.tile([P, DIAG_TOT], F32, tag="mm", bufs=2)
                    esd = work.tile([P, DIAG_TOT], MMDT, tag="esd", bufs=4)
                    ps1 = psum.tile([P, 256], F32, tag="proj", bufs=2)
                    es1 = work.tile([P, 256], MMDT, tag="es1", bufs=4)
                    for jj in range(4):
                        j = 4 * r + jj
                        if jj == 2:
                            sv = ps1[:, :]
                        else:
                            poff, wd = DIAG_PACK[jj]
                            sv = psd[:, poff:poff + wd]
                        nc.tensor.matmul(
                            sv,
                            KTd[:, j * P:(j + 1) * P],
                            QTd[:, r * W + jj * P:(r + 1) * W],
                            start=True, stop=True,
                        )
                    nc.scalar.activation(esd[:, 0:W], psd[:, 0:W], Exp,
                                         scale=0.125)
                    nc.scalar.activation(esd[:, W:], psd[:, W:], Exp,
                                         scale=0.125)
                    nc.scalar.activation(es1[:], ps1[:], Exp, scale=0.125)
                    # zero the invalid (q < k) half of each diagonal 128x128
                    # block post-exp, on the otherwise-idle GPSIMD engine
                    for jj in range(4):
                        ev = es1[:, 0:P] if jj == 2 else (
                            esd[:, DIAG_PACK[jj][0]:DIAG_PACK[jj][0] + P])
                        nc.gpsimd.affine_select(
                            out=ev, in_=ev,
                            compare_op=mybir.AluOpType.is_ge,
                            fill=0.0, base=0,
                            pattern=[[1, P]], channel_multiplier=-1,
                        )
                    for jj in range(4):
                        j = 4 * r + jj
                        if jj == 2:
                            rhs = es1[:, :]
                        else:
                            poff, wd = DIAG_PACK[jj]
                            rhs = esd[:, poff:poff + wd]
                        nc.tensor.matmul(
                            po[:, jj * P:],
                            Vb[:, j, :],
                            rhs,
                            start=((r == 0 or last_r) and jj == 0),
                            stop=(not last_r and jj == 3),
                            skip_group_check=True,
                        )

                    if last_r:
                        do_pairs(first=False, last=True)

                    # ---- normalize + output (pipelined per n-tile) ---------
                    ot = work.tile([H + 1, W], F32, tag="ot", bufs=4)
                    nc.vector.tensor_copy(out=ot[:], in_=po[:])
                    pf = psum.tile([P, 4, H + 1], F32, tag="proj", bufs=2)
                    for i in range(4):
                        nt = r * 4 + i
                        nc.tensor.transpose(
                            pf[:, i, :], ot[:, i * P:(i + 1) * P],
                            ident[:H + 1, :H + 1],
                        )
                        rs = work.tile([P, 1], F32, tag="rs",
                                       name=f"rs_{rep}_{nt}")
                        nc.vector.reciprocal(rs[:], pf[:, i, H:H + 1])
                        ob = work.tile([P, H], F32, tag="ob",
                                       name=f"ob_{rep}_{nt}", bufs=4)
                        nc.vector.tensor_scalar_mul(
                            ob[:], pf[:, i, 0:H], rs[:]
                        )
                        nc.sync.dma_start(
                            out=out_d[nt * P:(nt + 1) * P, :],
                            in_=ob[:],
                        )

    nc.compile()
    return nc


_NC_CACHE = {}


def _get_nc(reps=1):
    if reps not in _NC_CACHE:
        _NC_CACHE[reps] = build_kernel(reps)
    return _NC_CACHE[reps]


def kernel(input, Wq, Wk, Wv, **_unused):
    if BF16_X:
        import ml_dtypes
        input = np.ascontiguousarray(
            np.asarray(input).astype(ml_dtypes.bfloat16))
    else:
        input = np.ascontiguousarray(np.asarray(input, dtype=np.float32))
    Wq = np.ascontiguousarray(np.asarray(Wq, dtype=np.float32))
    Wk = np.ascontiguousarray(np.asarray(Wk, dtype=np.float32))
    Wv = np.ascontiguousarray(np.asarray(Wv, dtype=np.float32))
    B = input.shape[0]
    assert B == 8 and input.shape[1] == N and input.shape[2] == D

    nc = _get_nc()
    in_maps = [
        {"x": input[b], "Wq": Wq, "Wk": Wk, "Wv": Wv} for b in range(B)
    ]
    res = run_bass_kernel_spmd(nc, in_maps, core_ids=list(range(B)))
    return np.stack([res.results[b]["out"] for b in range(B)], axis=0)



# revision 3
# speedup vs baseline: 1.3414x; 1.3414x over previous
"""Bass/Tile kernel for a single causal attention head on 8 trn2 NeuronCores.

Problem: input [8, 2048, 768], Wq/Wk/Wv [768, 64] ->
  O = softmax(causal(Q K^T)/sqrt(64)) V, per batch.  [8, 2048, 64]

Sharding: data-parallel over batch; core b handles batch b. Weights replicated.

Per-core dataflow:
  1. x [2048, 768] -> PE-transpose (exact fp32) -> xT [768, 2048] in SBUF.
  2. Projections with concatenated stationary weights, so one 128-wide
     matmul yields two 64-row outputs:
       [Wq|Wk]^T xT -> psum [128, 512]: rows 0-63 Q^T, rows 64-127 K^T
       [Wv|Wq]^T xT -> psum [128, 512]: rows 0-63 V^T, rows 64-127 Q^T (dup)
     One full-width copy each into QK_sb / VQ_sb.  V^T tiles are
     PE-transposed back into V_aug tiles [128, 65] whose ones column makes
     the O^T matmul emit softmax denominators for free (row 64).
  3. For each 512-wide query range r:
       full k-tiles (j < 4r) in pairs: S^T pair-psum [128, 2x512] with
         lhsT=K^T_j, rhs=Q^T (both on partitions 64-127); one exp per pair
         (ScalarE, 1/8 scale fused).
       diagonal k-tiles j=4r..4r+3, widths 512/384/256/128: three packed
         into a [128, 1024] psum + one [128, 256]; after exp, the invalid
         (q < k) halves of the diagonal 128x128 blocks are zeroed by
         GPSIMD affine_select on SBUF.
       O^T [65, 512] accumulates V_aug_j^T @ expS^T over j in PSUM.
  4. O^T+rowsum PE-transposed back to [128, 65]; out = O * (1/rowsum); DMA.

Matmul inputs are stored as float32r (single-pass full-rate fp32 PE mode;
producing copies round to f32r precision; ~3e-4 scale-relative output
error).  ATTN_MM_DTYPE=float32 selects the exact two-pass fp32 mode
(4 cyc/row, ~2.4e-6 error).  ATTN_X=bf16 selects a bf16 x-path (host-cast
+ xbar DMA transpose, ~2.4e-3 error).
"""

import os
import numpy as np

import concourse.tile as tile
from concourse import bacc, mybir
from concourse.bass_utils import run_bass_kernel_spmd
from concourse.masks import make_identity

P = 128
N = 2048
D = 768
H = 64
NT = N // P   # 16 n-tiles
DT = D // P   # 6 d-tiles
W = 512       # q-range width
QR = N // W   # 4 q-ranges
F32 = mybir.dt.float32

_MM = os.environ.get("ATTN_MM_DTYPE", "bfloat16")
MMDT = {
    "float32r": mybir.dt.float32r,
    "float32": F32,
    "bfloat16": mybir.dt.bfloat16,
}[_MM]
MM_F32R = _MM == "float32r"
# bf16 x-path: host-cast x to bf16, xbar DMA-transpose straight from DRAM
# (removes all PE transposes + PSUM->SBUF copies); projections run in bf16.
BF16_X = os.environ.get("ATTN_X", "bf16") == "bf16"
XDT = mybir.dt.bfloat16 if BF16_X else MMDT

# Three of the four diagonal tiles (widths 512, 384, 128) pack into one
# [128, 1024] psum (bank0: 512 | bank1: 384+128); the 256-wide one (jj=2)
# gets its own narrow tile.  No matmul output crosses a 512-col bank.
DIAG_PACK = {0: (0, 512), 1: (512, 384), 3: (896, 128)}  # jj -> (off, width)
DIAG_TOT = 1024


def build_kernel(reps=1):
    nc = bacc.Bacc(name="attn_head")
    x_d = nc.dram_tensor("x", [N, D], XDT if BF16_X else F32,
                         kind="ExternalInput")
    wq_d = nc.dram_tensor("Wq", [D, H], F32, kind="ExternalInput")
    wk_d = nc.dram_tensor("Wk", [D, H], F32, kind="ExternalInput")
    wv_d = nc.dram_tensor("Wv", [D, H], F32, kind="ExternalInput")
    out_d = nc.dram_tensor("out", [N, H], F32, kind="ExternalOutput")

    Exp = mybir.ActivationFunctionType.Exp

    with tile.TileContext(nc) as tc:
        with (
            tc.tile_pool(name="persist", bufs=1) as persist,
            tc.tile_pool(name="xload", bufs=16) as xload,
            tc.tile_pool(name="work", bufs=3) as work,
            tc.tile_pool(name="psum", bufs=1, space="PSUM") as psum,
        ):
            # warm the ACT exp table while DMAs run
            dummy = persist.tile([P, 1], F32)
            nc.vector.memset(dummy[:], 0.0)
            nc.scalar.activation(dummy[:], dummy[:], Exp)

            ident = persist.tile([P, P], F32)
            make_identity(nc, ident[:])

            ones_col = persist.tile([P, 1], F32)
            nc.vector.memset(ones_col[:], 1.0)

            if MMDT != F32:
                ident_r = persist.tile([P, P], MMDT)
                nc.vector.tensor_copy(out=ident_r[:], in_=ident[:])
            else:
                ident_r = ident

            xT = persist.tile([P, DT, N], XDT)      # x^T: [d%128, d//128, n]
            QK_sb = persist.tile([P, N], MMDT)      # rows 0-63 Q^T, 64-127 K^T
            VQ_sb = persist.tile([P, N], MMDT)      # rows 0-63 V^T, 64-127 Q^T
            Vb = persist.tile([P, NT, H + 1], MMDT)  # V tiles + ones col
            nc.vector.tensor_copy(
                out=Vb[:, :, H],
                in_=ones_col[:, 0].to_broadcast((P, NT)),
            )

            # weights as [128, 6, 64]: partition = d%128, tile = d//128.
            # Concatenated pairs [Wq|Wk] and [Wv|Wq] make 128-wide stationary
            # operands: one projection matmul computes two 64-row outputs.
            w_raw = {}
            w_dma = []
            for wname, dram in (("q", wq_d), ("k", wk_d), ("v", wv_d)):
                w_raw[wname] = persist.tile([P, DT, H], F32,
                                            name=f"wraw_{wname}")
                w_dma.append((w_raw[wname], dram))
            w_qk = persist.tile([P, DT, 2 * H], XDT)
            w_vq = persist.tile([P, DT, 2 * H], XDT)

            def load_weights():
                for w_t, dram in w_dma:
                    nc.sync.dma_start(
                        out=w_t[:],
                        in_=dram[:, :].rearrange("(t p) h -> p t h", p=P),
                    )
                nc.vector.tensor_copy(out=w_qk[:, :, 0:H], in_=w_raw["q"][:])
                nc.vector.tensor_copy(out=w_qk[:, :, H:], in_=w_raw["k"][:])
                nc.vector.tensor_copy(out=w_vq[:, :, 0:H], in_=w_raw["v"][:])
                nc.vector.tensor_copy(out=w_vq[:, :, H:], in_=w_raw["q"][:])

            for rep in range(reps):
                # ---- x load + transpose to xT ------------------------------
                if BF16_X:
                    # x arrives bf16: xbar DMA-transpose straight from DRAM,
                    # one [2048, 128] -> [128, 2048] transpose per d-block
                    for d_i in range(DT):
                        nc.sync.dma_start(
                            out=xT[:, d_i, :],
                            in_=x_d[:, d_i * P:(d_i + 1) * P],
                            transpose=True,
                        )
                    if rep == 0:
                        load_weights()
                else:
                    # first group (tiles 0-3) split into d-halves so the
                    # first PE transposes start after ~4 small DMAs
                    xts = []
                    for nt in range(NT):
                        xt = xload.tile([P, D], F32, tag="x",
                                        name=f"x_{rep}_{nt}")
                        xts.append(xt)
                    for nt in range(4):
                        nc.sync.dma_start(
                            out=xts[nt][:, 0:D // 2],
                            in_=x_d[nt * P:(nt + 1) * P, 0:D // 2])
                    for nt in range(4):
                        nc.sync.dma_start(
                            out=xts[nt][:, D // 2:],
                            in_=x_d[nt * P:(nt + 1) * P, D // 2:])
                    if rep == 0:
                        load_weights()
                    for nt in range(4, NT):
                        nc.sync.dma_start(
                            out=xts[nt][:, 0:D // 2],
                            in_=x_d[nt * P:(nt + 1) * P, 0:D // 2])
                        nc.sync.dma_start(
                            out=xts[nt][:, D // 2:],
                            in_=x_d[nt * P:(nt + 1) * P, D // 2:])

                    for g in range(QR):
                        for dp in range(DT // 2):
                            pt = psum.tile([P, 2, W], F32, tag="mm", bufs=2)
                            for u in range(2):
                                d_i = 2 * dp + u
                                for i in range(4):
                                    nc.tensor.transpose(
                                        pt[:, u, i * P:(i + 1) * P],
                                        xts[g * 4 + i][:,
                                                       d_i * P:(d_i + 1) * P],
                                        ident[:],
                                    )
                            base = g * W
                            if (g * 3 + dp) % 4 == 3:
                                nc.scalar.copy(
                                    out=xT[:, 2 * dp:2 * dp + 2,
                                           base:base + W],
                                    in_=pt[:])
                            else:
                                nc.vector.tensor_copy(
                                    out=xT[:, 2 * dp:2 * dp + 2,
                                           base:base + W],
                                    in_=pt[:])

                # ---- projections (packed via concatenated weights) ---------
                for r in range(QR):
                    ns = slice(r * W, (r + 1) * W)
                    pqk = psum.tile([P, W], F32, tag="proj", bufs=2)
                    pvq = psum.tile([P, W], F32, tag="proj", bufs=2)
                    for d_i in range(DT):
                        kw = dict(start=(d_i == 0), stop=(d_i == DT - 1))
                        rhs = xT[:, d_i, ns]
                        nc.tensor.matmul(pqk[:], w_qk[:, d_i], rhs, **kw)
                        nc.tensor.matmul(pvq[:], w_vq[:, d_i], rhs, **kw)
                    nc.vector.tensor_copy(out=QK_sb[:, ns], in_=pqk[:])
                    nc.vector.tensor_copy(out=VQ_sb[:, ns], in_=pvq[:])
                    # V^T tiles -> V_aug [128, 65] per k-tile (batched copy)
                    pvt = psum.tile([P, 4, H], MMDT, tag="proj", bufs=2)
                    for i in range(4):
                        nc.tensor.transpose(
                            pvt[:, i, :],
                            VQ_sb[0:H, r * W + i * P:r * W + (i + 1) * P],
                            ident_r[:H, :H],
                        )
                    nc.vector.tensor_copy(out=Vb[:, 4 * r:4 * r + 4, 0:H],
                                          in_=pvt[:])

                # ---- attention per q-range ---------------------------------
                KTd = QK_sb[H:P, :]   # K^T on partitions 64-127
                QTd = VQ_sb[H:P, :]   # Q^T duplicate on partitions 64-127
                for r in range(QR):
                    po = psum.tile([H + 1, W], F32, tag="po", bufs=2)
                    qs = slice(r * W, (r + 1) * W)
                    last_r = (r == QR - 1)

                    def do_pairs(first, last):
                        for jp in range(2 * r):
                            ps2 = psum.tile([P, 2, W], F32, tag="mm", bufs=2,
                                            name=f"ps2_{rep}_{r}_{jp}")
                            es2 = work.tile([P, 2, W], MMDT, tag="es2", bufs=6,
                                            name=f"es2_{rep}_{r}_{jp}")
                            for u in range(2):
                                j = 2 * jp + u
                                nc.tensor.matmul(
                                    ps2[:, u, :], KTd[:, j * P:(j + 1) * P],
                                    QTd[:, qs], start=True, stop=True,
                                )
                            nc.scalar.activation(es2[:], ps2[:], Exp,
                                                 scale=0.125)
                            for u in range(2):
                                j = 2 * jp + u
                                nc.tensor.matmul(
                                    po[:], Vb[:, j, :], es2[:, u, :],
                                    start=(first and j == 0),
                                    stop=(last and jp == 2 * r - 1 and u == 1),
                                    skip_group_check=True,
                                )

                    if not last_r:
                        do_pairs(first=True, last=False)

                    # diagonal k-tiles: 3 packed in [128, 1024] + 1 [128, 256]
                    psd = psum.tile([P, DIAG_TOT], F32, tag="mm", bufs=2)
                    esd = work.tile([P, DIAG_TOT], MMDT, tag="esd", bufs=4)
                    ps1 = psum.tile([P, 256], F32, tag="proj", bufs=2)
                    es1 = work.tile([P, 256], MMDT, tag="es1", bufs=4)
                    for jj in range(4):
                        j = 4 * r + jj
                        if jj == 2:
                            sv = ps1[:, :]
                        else:
                            poff, wd = DIAG_PACK[jj]
                            sv = psd[:, poff:poff + wd]
                        nc.tensor.matmul(
                            sv,
                            KTd[:, j * P:(j + 1) * P],
                            QTd[:, r * W + jj * P:(r + 1) * W],
                            start=True, stop=True,
                        )
                    nc.scalar.activation(esd[:, 0:W], psd[:, 0:W], Exp,
                                         scale=0.125)
                    nc.scalar.activation(esd[:, W:], psd[:, W:], Exp,
                                         scale=0.125)
                    nc.scalar.activation(es1[:], ps1[:], Exp, scale=0.125)
                    # zero the invalid (q < k) half of each diagonal 128x128
                    # block post-exp, on the otherwise-idle GPSIMD engine
                    for jj in range(4):
                        ev = es1[:, 0:P] if jj == 2 else (
                            esd[:, DIAG_PACK[jj][0]:DIAG_PACK[jj][0] + P])
                        nc.gpsimd.affine_select(
                            out=ev, in_=ev,
                            compare_op=mybir.AluOpType.is_ge,
                            fill=0.0, base=0,
                            pattern=[[1, P]], channel_multiplier=-1,
                        )
                    for jj in range(4):
                        j = 4 * r + jj
                        if jj == 2:
                            rhs = es1[:, :]
                        else:
                            poff, wd = DIAG_PACK[jj]
                            rhs = esd[:, poff:poff + wd]
                        nc.tensor.matmul(
                            po[:, jj * P:],
                            Vb[:, j, :],
                            rhs,
                            start=((r == 0 or last_r) and jj == 0),
                            stop=(not last_r and jj == 3),
                            skip_group_check=True,
                        )

                    if last_r:
                        do_pairs(first=False, last=True)

                    # ---- normalize + output (pipelined per n-tile) ---------
                    ot = work.tile([H + 1, W], F32, tag="ot", bufs=4)
                    nc.vector.tensor_copy(out=ot[:], in_=po[:])
                    pf = psum.tile([P, 4, H + 1], F32, tag="proj", bufs=2)
                    for i in range(4):
                        nt = r * 4 + i
                        nc.tensor.transpose(
                            pf[:, i, :], ot[:, i * P:(i + 1) * P],
                            ident[:H + 1, :H + 1],
                        )
                        rs = work.tile([P, 1], F32, tag="rs",
                                       name=f"rs_{rep}_{nt}")
                        nc.vector.reciprocal(rs[:], pf[:, i, H:H + 1])
                        ob = work.tile([P, H], F32, tag="ob",
                                       name=f"ob_{rep}_{nt}", bufs=4)
                        nc.vector.tensor_scalar_mul(
                            ob[:], pf[:, i, 0:H], rs[:]
                        )
                        nc.sync.dma_start(
                            out=out_d[nt * P:(nt + 1) * P, :],
                            in_=ob[:],
                        )

    nc.compile()
    return nc


_NC_CACHE = {}


def _get_nc(reps=1):
    if reps not in _NC_CACHE:
        _NC_CACHE[reps] = build_kernel(reps)
    return _NC_CACHE[reps]


def kernel(input, Wq, Wk, Wv, **_unused):
    if BF16_X:
        import ml_dtypes
        input = np.ascontiguousarray(
            np.asarray(input).astype(ml_dtypes.bfloat16))
    else:
        input = np.ascontiguousarray(np.asarray(input, dtype=np.float32))
    Wq = np.ascontiguousarray(np.asarray(Wq, dtype=np.float32))
    Wk = np.ascontiguousarray(np.asarray(Wk, dtype=np.float32))
    Wv = np.ascontiguousarray(np.asarray(Wv, dtype=np.float32))
    B = input.shape[0]
    assert B == 8 and input.shape[1] == N and input.shape[2] == D

    nc = _get_nc()
    in_maps = [
        {"x": input[b], "Wq": Wq, "Wk": Wk, "Wv": Wv} for b in range(B)
    ]
    res = run_bass_kernel_spmd(nc, in_maps, core_ids=list(range(B)))
    return np.stack([res.results[b]["out"] for b in range(B)], axis=0)

